# revision 1
# baseline (speedup 1.0000x reference)
"""Trainium2 Bass kernel for nn_DecoderND_39058432590521.

Sharding: data-parallel over batch B=16 across 8 NeuronCores (B=2 rows per
core, zero collectives). Each core runs the full 2-layer additive-attention
GRU scan for its 2 batch rows with the active layer's weights SBUF-resident
in fp16 (fp32 PSUM accumulation, fp32 recurrent state), using
batch-stationary column-tiled matmuls (4 concurrent weight streams through
the PE array), then computes its batch slice of the output projection.
Host concatenates per-core outputs.
"""
import sys
sys.path.insert(0, '/opt/trn_rl_repo')
import numpy as np

import concourse.bass as bass
import concourse.mybir as mybir
import concourse.tile as tile
import bass_rust
from concourse.bass_utils import run_bass_kernel_spmd

F16 = mybir.dt.float16
F32 = mybir.dt.float32
AF = mybir.ActivationFunctionType


# ---------------------------------------------------------------------------
# This toolchain's walrus rejects >1 sync wait on TPB_CTRL instructions; the
# stock TileContext exit drain carries one wait per live processor. Split the
# waits one-per-nop ahead of a bare drain.
def _patched_drain_and_barrier(self, tick_clock, wait_clock):
    from concourse.tile import ScopedClock
    probe = self.nc.sync.nop(nofuse=True)
    wait_clock.add_sem_waits(probe.ins, ScopedClock({None: tick_clock.global_clock}))
    waits = list(probe.ins.sync_info.on_wait)
    probe.ins.sync_info = bass_rust.SyncInfo(on_wait=waits[:1], on_update=[])
    for w in waits[1:]:
        n = self.nc.sync.nop(nofuse=True)
        n.ins.sync_info = bass_rust.SyncInfo(on_wait=[w], on_update=[])
    self.nc.sync.drain()
    self.nc.all_engine_barrier()
    assert self.sems is not None
    popped = self.nc._tile_sem_poison_stack.pop()
    assert popped is self._sem_poison
    self.nc.clear_and_free_semaphores(list(self.sems.allocated().values()))
    self.nc.all_engine_barrier()


tile.TileContext._drain_and_barrier = _patched_drain_and_barrier


# Split any instruction carrying more than one sync wait: hoist the extra
# waits onto same-engine NOPs inserted immediately before it (this walrus
# build rejects multi-wait sync setup on several instruction classes).
def _split_excess_waits(nc, limit=1):
    mknop_cache = {}

    def mknop(engine):
        eng = nc.engines[engine]
        inst = eng.nop(nofuse=True)
        # remove from wherever the builder appended it
        for bb in nc.main_func.blocks:
            lst = bb.instructions
            if lst and lst[-1].name == inst.ins.name:
                bb.instructions = lst[:-1]
                break
        return inst.ins

    for bb in nc.main_func.blocks:
        changed = False
        out = []
        for inst in bb.instructions:
            si = inst.sync_info
            waits = list(si.on_wait) if si is not None else []
            if len(waits) > limit:
                for w in waits[:-limit]:
                    nop = mknop(inst.engine)
                    nop.sync_info = bass_rust.SyncInfo(on_wait=[w], on_update=[])
                    out.append(nop)
                inst.sync_info = bass_rust.SyncInfo(on_wait=waits[-limit:],
                                                    on_update=list(si.on_update))
                changed = True
            out.append(inst)
        if changed:
            bb.instructions = out


_orig_sched = tile.TileContext.schedule_and_allocate


def _patched_sched(self, *a, **k):
    r = _orig_sched(self, *a, **k)
    _split_excess_waits(self.nc)
    return r


tile.TileContext.schedule_and_allocate = _patched_sched


class Cfg:
    def __init__(self, T=64, V=32000, NG=4, debug_h=False):
        self.B = 2
        self.H, self.E, self.T, self.TX, self.V = 1024, 512, T, 128, V
        self.NG = NG
        self.KH = self.H // 128
        self.K2H = 2 * self.H // 128
        self.QW = self.H // NG
        self.GW = 3 * self.H // NG
        self.VC = 512
        self.debug_h = debug_h


FULL = Cfg()


def build_kernel(c: Cfg):
    nc = bass.Bass(target_bir_lowering=False)
    B, H, E, T, TX, V, NG = c.B, c.H, c.E, c.T, c.TX, c.V, c.NG
    KH, K2H, QW, GW = c.KH, c.K2H, c.QW, c.GW
    H3, BT, KB = 3 * H, B * T, 2 * c.KH
    assert B == 2 and TX == 128

    def dram_in(name, shape, dt=F16):
        return nc.dram_tensor(name, shape, dt, kind="ExternalInput")

    xT_d = dram_in("xT", [E, BT])
    WaT_d = dram_in("WaT", [H, H])
    UaT_d = dram_in("UaT", [2 * H, H])
    va_d = dram_in("va", [128, KH])
    ones16_d = dram_in("ones16", [1, 256])
    ones32_d = dram_in("ones32", [1, 128], F32)
    uab_d = dram_in("uab", [128, H // 128], F32)
    WixT0_d = dram_in("WixT0", [E, H3])
    WixT1_d = dram_in("WixT1", [H, H3])
    WicT_d = [dram_in(f"WicT{l}", [2 * H, H3]) for l in range(2)]
    WhhT_d = [dram_in(f"WhhT{l}", [H, H3]) for l in range(2)]
    gxb_d = [dram_in(f"gxb{l}", [128, 3 * H // 128], F32) for l in range(2)]
    bhhn_d = [dram_in(f"bhhn{l}", [128, 2 * H // 128], F32) for l in range(2)]
    keysT_d = [dram_in(f"keysT{l}", [2 * H, B * TX]) for l in range(2)]
    keys_d = [dram_in(f"keys{l}", [TX, B * 2 * H]) for l in range(2)]
    iW_d = [dram_in(f"iW{l}", [H, H]) for l in range(2)]
    outwT_d = dram_in("outwT", [H, V])
    outb_d = dram_in("outb", [1, V])

    out_d = nc.dram_tensor("out", [BT, V], F32, kind="ExternalOutput")
    if c.debug_h:
        hdbg = [nc.dram_tensor(f"hdbg{l}", [128, T * KB], F16,
                               kind="ExternalOutput") for l in range(2)]
        dbg = {}
        for nm, shp, dt in [("dq", [128, KB], F32), ("dA", [128, 4 * B * 128], F16),
                            ("dw", [B, 128], F16), ("dZ", [1, B], F32),
                            ("dcT", [128, B * K2H], F16), ("dg", [128, 4 * KB], F32),
                            ("dgx", [128, 3 * KB], F32), ("dgab", [128, GW + QW], F16),
                            ("dA0", [128, 4 * B * 128], F16),
                            ("dpsc", [128, B * 128], F32)]:
            dbg[nm] = nc.dram_tensor(nm, shp, dt, kind="ExternalOutput")
    gx_dram = nc.dram_tensor("gx_scratch", [128, 3 * KH * BT], F16)

    def r_kt(d, inner=128):
        return d.ap().rearrange("(kt k) n -> k kt n", k=inner)

    with tile.TileContext(nc) as tc:
        import contextlib
        with contextlib.ExitStack() as ctx:
            wpool = ctx.enter_context(tc.tile_pool(name="wsmall", bufs=1))
            spool = ctx.enter_context(tc.tile_pool(name="state", bufs=1))

            va_sb = wpool.tile([128, KH], F16)
            ones16 = wpool.tile([1, 256], F16)
            ones32 = wpool.tile([1, 128], F32)
            id1 = wpool.tile([1, 1], F16)
            bhhn = wpool.tile([128, KB], F32)

            UaK = spool.tile([128, KH, B * 128], F16)
            keys_sb = spool.tile([128, B, 2 * H], F16)
            hsT = [spool.tile([128, KH, T, B], F16, tag=f"hsT{l}", name=f"hsT{l}")
                   for l in range(2)]
            h32 = spool.tile([128, KB], F32)
            h16i = spool.tile([128, KB], F16)
            A16 = spool.tile([128, 4 * B * 128], F16)      # half of the h-tiles
            q_sb = spool.tile([128, QW], F16)
            qT32 = spool.tile([128, KB], F32)
            c_sb = spool.tile([128, B, 512], F16)
            cT16 = spool.tile([128, B, K2H], F16)
            wT16 = spool.tile([128, B], F16)
            w2row = spool.tile([1, B, 128], F16)
            Zrow = spool.tile([1, B], F32)
            rZrow = spool.tile([1, B], F32)
            gAB_sb = spool.tile([128, GW + QW], F16)
            g48f = spool.tile([128, 4 * KB], F32)
            gxt16 = spool.tile([128, 3 * KB], F16)
            gxt = spool.tile([128, 3 * KB], F32)
            id128 = spool.tile([128, 128], F16)
            rz = spool.tile([128, 2 * KB], F32)
            nin = spool.tile([128, KB], F32)
            ngate = spool.tile([128, KB], F32)
            tmpg = spool.tile([128, KB], F32)

            from concourse.masks import make_identity
            nc.gpsimd.memset(ones16[:], 1.0)
            nc.gpsimd.memset(id1[:], 1.0)
            make_identity(nc, id128[:])
            nc.sync.dma_start(ones32[:], ones32_d[:])
            nc.sync.dma_start(va_sb[:], va_d[:])
            nc.sync.dma_start(bhhn[:], bhhn_d[0][:])

            # ---------------- per-layer prep ----------------
            def prep_layer(l, pp, pspool):
                UaT_sb = pp.tile([128, K2H, H], F16, tag="UaT")
                keysT_sb = pp.tile([128, K2H, B * TX], F16, tag="keysT")
                iW_sb = pp.tile([128, KH, H], F16, tag="iW")
                uab_sb = pp.tile([128, KH], F32, tag="uab")
                nc.sync.dma_start(UaT_sb[:], r_kt(UaT_d))
                nc.sync.dma_start(keysT_sb[:], r_kt(keysT_d[l]))
                nc.sync.dma_start(iW_sb[:], r_kt(iW_d[l]))
                nc.sync.dma_start(uab_sb[:], uab_d[:])
                for ht in range(KH):
                    pu = pspool.tile([128, B * TX], F32, tag="pu")
                    for kt in range(K2H):
                        nc.tensor.matmul(pu[:], UaT_sb[:, kt, ht * 128:(ht + 1) * 128],
                                         keysT_sb[:, kt, :], start=(kt == 0),
                                         stop=(kt == K2H - 1))
                    nc.vector.tensor_scalar_add(UaK[:, ht, :], pu[:],
                                                uab_sb[:, ht:ht + 1])
                for ht in range(KH):
                    ps0 = pspool.tile([128, B], F32, tag="ps0")
                    for kt in range(KH):
                        rhs = keysT_sb[:, KH + kt, :].rearrange(
                            "k (b t) -> k b t", b=B)[:, :, 0]
                        nc.tensor.matmul(ps0[:], iW_sb[:, kt, ht * 128:(ht + 1) * 128],
                                         rhs, start=(kt == 0), stop=(kt == KH - 1))
                    nc.vector.tensor_copy(h32[:, ht * 2:(ht + 1) * 2], ps0[:])

            def gx_compute(l, rhsT, KD, WixT_t, pp, pspool):
                # transposed: gxT block (pc, kt) = WixT-cols.T @ xT  [128, BT]
                gxb_sb = pp.tile([128, 3 * KH], F32, tag="gxb")
                gstage = pp.tile([128, BT], F16, tag="gstage")
                nc.sync.dma_start(gxb_sb[:], gxb_d[l][:])
                for pcg in range(3):
                    for kt in range(KH):
                        j = (kt // 2) * 6 + pcg * 2 + (kt % 2)
                        pgx = pspool.tile([128, BT], F32, tag="pgx")
                        for kd in range(KD):
                            nc.tensor.matmul(pgx[:], WixT_t[:, kd, j * 128:(j + 1) * 128],
                                             rhsT(kd), start=(kd == 0),
                                             stop=(kd == KD - 1))
                        blk = pcg * KH + kt
                        nc.vector.tensor_scalar_add(gstage[:], pgx[:],
                                                    gxb_sb[:, blk:blk + 1])
                        nc.sync.dma_start(
                            gx_dram.ap().rearrange(
                                "p (blk tb) -> p blk tb", blk=3 * KH)[:, blk, :],
                            gstage[:])

            # ---------------- the scan ----------------
            def scan_layer(l, WaT, WicT, WhhT, ps):
                pq = ps.tile([128, QW], F32, tag="pq", name=f"pq{l}")
                pg = ps.tile([128, GW + QW], F32, tag="pg", name=f"pg{l}")
                psc = ps.tile([128, B, 512], F32, tag="psc", name=f"psc{l}")
                pc = ps.tile([128, B, 512], F32, tag="pc", name=f"pc{l}")
                ptr = ps.tile([128, 8, 128], F16, tag="ptr", name=f"ptr{l}")
                # dummy-init full tiles so evacuation reads see owned data
                for nnn in range(0, QW, 256):
                    nc.tensor.matmul(pq[:, nnn:nnn + 256], ones16[0:1, 0:128],
                                     ones16[0:1, 0:256], start=True, stop=True)
                for nnn in range(0, GW + QW, 256):
                    nc.tensor.matmul(pg[:, nnn:nnn + 256], ones16[0:1, 0:128],
                                     ones16[0:1, 0:256], start=True, stop=True)
                pcf = pc[:].rearrange("p b x -> p (b x)")
                for nnn in range(0, B * 512, 256):
                    nc.tensor.matmul(pcf[:, nnn:nnn + 256], ones16[0:1, 0:128],
                                     ones16[0:1, 0:256], start=True, stop=True)
                nc.vector.tensor_copy(h16i[:], h32[:])
                for t in range(T):
                    def hsl(kt, _t=t):
                        if _t == 0:
                            return h16i[:, kt * 2:kt * 2 + 2]
                        return hsT[l][:, kt, _t - 1, :]
                    # q (batch-stationary, col-tiled)
                    for kt in range(KH):
                        for g in range(NG):
                            nc.tensor.matmul(
                                pq[32 * g:32 * g + 2, :], hsl(kt),
                                WaT[:, kt, g * QW:(g + 1) * QW],
                                start=(kt == 0), stop=(kt == KH - 1),
                                tile_position=(0, 32 * g), skip_group_check=True)
                    # gh into gates psum: rz -> [0:2QW], ghn -> [GW:GW+QW]
                    for kt in range(KH):
                        for g in range(NG):
                            nc.tensor.matmul(
                                pg[32 * g:32 * g + 2, 0:2 * QW],
                                hsl(kt),
                                WhhT[:, kt, g * GW:g * GW + 2 * QW],
                                start=(kt == 0), stop=False,
                                tile_position=(0, 32 * g), skip_group_check=True)
                            nc.tensor.matmul(
                                pg[32 * g:32 * g + 2, GW:GW + QW],
                                hsl(kt),
                                WhhT[:, kt, g * GW + 2 * QW:(g + 1) * GW],
                                start=(kt == 0), stop=(kt == KH - 1),
                                tile_position=(0, 32 * g), skip_group_check=True)
                    # qT: evac + PE transpose + strided gather
                    nc.scalar.copy(q_sb[:], pq[:])
                    for kl in range(2):
                        nc.tensor.transpose(ptr[:, kl, :],
                                            q_sb[:, kl * 128:(kl + 1) * 128],
                                            id128[:])
                    # qT32[p, (2g+kl)*2+b] = ptr[p, kl, 32g+b]
                    gsrc = ptr[:, 0:2, :].rearrange("p kl (g b) -> p kl g b", b=32)[
                        :, :, :, 0:2]
                    gdst = qT32[:].rearrange("p (g kl b) -> p kl g b", kl=2, g=NG)
                    nc.vector.tensor_copy(gdst, gsrc)
                    # attention in two h-tile halves
                    for half in range(2):
                        for hl in range(4):
                            ht = 4 * half + hl
                            for b in range(B):
                                nc.vector.tensor_scalar_add(
                                    A16[:, hl * 256 + b * 128:hl * 256 + (b + 1) * 128],
                                    UaK[:, ht, b * 128:(b + 1) * 128],
                                    qT32[:, ht * 2 + b:ht * 2 + b + 1])
                        nc.scalar.activation(A16[:], A16[:], AF.Tanh)
                        if c.debug_h and t == 0 and l == 0 and half == 0:
                            nc.sync.dma_start(dbg["dA0"][:], A16[:])
                        for hl in range(4):
                            ht = 4 * half + hl
                            for b in range(B):
                                nc.tensor.matmul(
                                    psc[0:1, b, 0:128], va_sb[:, ht:ht + 1],
                                    A16[:, hl * 256 + b * 128:hl * 256 + (b + 1) * 128],
                                    start=(ht == 0), stop=(ht == KH - 1),
                                    skip_group_check=True)
                    for b in range(B):
                        nc.scalar.activation(w2row[0:1, b, :], psc[0:1, b, 0:128], AF.Exp,
                                             accum_out=Zrow[0:1, b:b + 1])
                    nc.vector.reciprocal(rZrow[:], Zrow[:])
                    for b in range(B):
                        nc.vector.tensor_scalar_mul(w2row[0:1, b, :],
                                                    w2row[0:1, b, :],
                                                    rZrow[0:1, b:b + 1])
                    for b in range(B):
                        nc.tensor.transpose(ptr[:, 7, 2 * b:2 * b + 1],
                                            w2row[0:1, b, :], id1[:])
                    nc.vector.tensor_copy(
                        wT16[:], ptr[:, 7, 0:4].rearrange(
                            "p (b o) -> p b o", o=2)[:, :, 0])
                    # c = w.T @ keys (unnormalized), col-tiled by f-chunk
                    for b in range(B):
                        for fc in range(4):
                            nc.tensor.matmul(
                                pc[32 * fc:32 * fc + 1, b, :], wT16[:, b:b + 1],
                                keys_sb[:, b, fc * 512:(fc + 1) * 512],
                                start=True, stop=True, tile_position=(0, 32 * fc),
                                skip_group_check=True)
                    nc.scalar.copy(c_sb[:], pc[:])
                    for b in range(B):
                        for kl in range(4):
                            nc.tensor.transpose(
                                ptr[:, b * 4 + kl, :],
                                c_sb[:, b, kl * 128:(kl + 1) * 128], id128[:])
                    # cTr[p, b, 4fc+kl] = ptr[p, b*4+kl, 32fc]
                    csrc = ptr[:, 0:8, :].rearrange(
                        "p (b kl) (fc r) -> p b fc kl r", b=B, r=32)[:, :, :, :, 0]
                    cdst = cT16[:, :, :].rearrange("p b (fc kl) -> p b fc kl", fc=4)
                    nc.vector.tensor_copy(cdst, csrc)
                    # gc into gates psum
                    for kt in range(K2H):
                        for g in range(NG):
                            nc.tensor.matmul(
                                pg[32 * g:32 * g + 2, 0:2 * QW], cT16[:, :, kt],
                                WicT[:, kt, g * GW:g * GW + 2 * QW],
                                start=False, stop=(kt == K2H - 1),
                                tile_position=(0, 32 * g), skip_group_check=True)
                            nc.tensor.matmul(
                                pg[32 * g:32 * g + 2, 2 * QW:3 * QW], cT16[:, :, kt],
                                WicT[:, kt, g * GW + 2 * QW:(g + 1) * GW],
                                start=(kt == 0), stop=(kt == K2H - 1),
                                tile_position=(0, 32 * g), skip_group_check=True)
                    # gates: evac + PE transposes + strided gathers
                    nc.vector.tensor_copy(gAB_sb[:], pg[:])
                    for j in range(8):
                        nc.tensor.transpose(ptr[:, j, :],
                                            gAB_sb[:, j * 128:(j + 1) * 128],
                                            id128[:])
                    # g48f[p, pc*16+(2g+kl)*2+b] = ptg[p, pc*2+kl, 32g+b]
                    for kl in range(2):
                        gsrc = ptr[:, :, :].rearrange(
                            "p (pc kl) (g b) -> p kl pc g b", kl=2, b=32)[
                            :, kl, :, :, 0:2]
                        gdst = g48f[:].rearrange(
                            "p (pc g kl b) -> p kl pc g b", pc=4, g=NG, kl=2)[:, kl]
                        nc.vector.tensor_copy(gdst, gsrc)
                    nc.sync.dma_start(
                        gxt16[:].rearrange("p (blk b) -> p blk b", b=B),
                        gx_dram.ap().rearrange(
                            "p (blk tb) -> p blk tb", blk=3 * KH)[:, :, 2 * t:2 * t + 2])
                    nc.vector.tensor_copy(gxt[:], gxt16[:])
                    # gates elementwise (fp32)
                    if c.debug_h and t == 0 and l == 0:
                        nc.sync.dma_start(dbg["dq"][:], qT32[:])
                        nc.sync.dma_start(dbg["dA"][:], A16[:])
                        nc.sync.dma_start(dbg["dw"][:], w2row[0, :, :])
                        nc.sync.dma_start(dbg["dZ"][:], Zrow[:])
                        nc.sync.dma_start(dbg["dcT"][:],
                                          cT16[:].rearrange("p b k -> p (b k)"))
                        nc.sync.dma_start(dbg["dg"][:], g48f[:])
                        nc.sync.dma_start(dbg["dgx"][:], gxt[:])
                        nc.sync.dma_start(dbg["dgab"][:], gAB_sb[:])
                    nc.vector.tensor_add(rz[:], g48f[:, 0:2 * KB], gxt[:, 0:2 * KB])
                    nc.scalar.activation(rz[:], rz[:], AF.Sigmoid)
                    nc.vector.tensor_add(tmpg[:], g48f[:, 3 * KB:4 * KB], bhhn[:])
                    nc.vector.tensor_mul(nin[:], rz[:, 0:KB], tmpg[:])
                    nc.vector.tensor_add(nin[:], nin[:], g48f[:, 2 * KB:3 * KB])
                    nc.vector.tensor_add(nin[:], nin[:], gxt[:, 2 * KB:3 * KB])
                    nc.scalar.activation(ngate[:], nin[:], AF.Tanh)
                    nc.vector.tensor_sub(tmpg[:], h32[:], ngate[:])
                    nc.vector.tensor_mul(tmpg[:], tmpg[:], rz[:, KB:2 * KB])
                    nc.vector.tensor_add(h32[:], ngate[:], tmpg[:])
                    nc.vector.tensor_copy(
                        hsT[l][:, :, t, :],
                        h32[:].rearrange("p (kt b) -> p kt b", b=B))
                if c.debug_h:
                    nc.sync.dma_start(
                        hdbg[l][:],
                        hsT[l][:, :, :, :].rearrange("p kt t b -> p (kt t b)"))

            # ================= phases =================
            with tc.tile_pool(name="prep0", bufs=1) as pp, \
                 tc.tile_pool(name="psA", bufs=1, space="PSUM") as psA:
                prep_layer(0, pp, psA)
                WixT0_sb = pp.tile([128, E // 128, H3], F16, tag="Wix")
                xT_sb = pp.tile([128, E // 128, BT], F16, tag="xTs")
                nc.sync.dma_start(WixT0_sb[:], r_kt(WixT0_d))
                nc.sync.dma_start(xT_sb[:], r_kt(xT_d))
                gx_compute(0, lambda kt: xT_sb[:, kt, :], E // 128, WixT0_sb, pp, psA)

            for l in range(2):
                if l == 1:
                    nc.sync.dma_start(bhhn[:], bhhn_d[1][:])
                    with tc.tile_pool(name="prep1", bufs=1) as pp, \
                         tc.tile_pool(name="psB", bufs=1, space="PSUM") as psB:
                        prep_layer(1, pp, psB)
                        WixT1_sb = pp.tile([128, KH, H3], F16, tag="Wix1")
                        nc.sync.dma_start(WixT1_sb[:], r_kt(WixT1_d))
                        gx_compute(1, lambda kt: hsT[0][:, kt, :, :].rearrange(
                                       "p t b -> p (t b)"),
                                   KH, WixT1_sb, pp, psB)
                with tc.tile_pool(name=f"bigw{l}", bufs=1) as bw, \
                     tc.tile_pool(name=f"psS{l}", bufs=1, space="PSUM") as ps:
                    WaT = bw.tile([128, KH, H], F16, tag="WaT")
                    WicT = bw.tile([128, K2H, H3], F16, tag="WicT")
                    WhhT = bw.tile([128, KH, H3], F16, tag="WhhT")
                    nc.sync.dma_start(WaT[:], r_kt(WaT_d))
                    nc.sync.dma_start(WicT[:], r_kt(WicT_d[l]))
                    nc.sync.dma_start(WhhT[:], r_kt(WhhT_d[l]))
                    nc.sync.dma_start(keys_sb[:],
                                      keys_d[l].ap().rearrange("t (b f) -> t b f", b=B))
                    scan_layer(l, WaT, WicT, WhhT, ps)

            # ---- output projection ----
            with tc.tile_pool(name="proj", bufs=3) as proj, \
                 tc.tile_pool(name="psP", bufs=2, space="PSUM") as psP:
                skipT = spool.tile([128, T * KB], F16, tag="skipT")
                nc.vector.tensor_add(
                    skipT[:],
                    hsT[0][:, :, :, :].rearrange("p kt t b -> p (kt t b)"),
                    hsT[1][:, :, :, :].rearrange("p kt t b -> p (kt t b)"))
                sk3 = skipT[:].rearrange("p (kt tb) -> p kt tb", kt=KH)
                NCH = (V + c.VC - 1) // c.VC
                for nci in range(NCH):
                    n0 = nci * c.VC
                    n1 = min(V, n0 + c.VC)
                    wchunk = proj.tile([128, KH, c.VC], F16, tag="wchunk")
                    nc.sync.dma_start(wchunk[:, :, 0:n1 - n0],
                                      r_kt(outwT_d)[:, :, n0:n1])
                    obc = proj.tile([1, c.VC], F16, tag="obc")
                    nc.sync.dma_start(obc[0:1, 0:n1 - n0], outb_d[0:1, n0:n1])
                    po = psP.tile([128, c.VC], F32, tag="pout")
                    for kt in range(KH):
                        nc.tensor.matmul(po[0:BT, 0:n1 - n0],
                                         sk3[:, kt, :],
                                         wchunk[:, kt, 0:n1 - n0],
                                         start=(kt == 0), stop=False)
                    nc.tensor.matmul(po[0:BT, 0:n1 - n0], ones16[0:1, 0:BT],
                                     obc[0:1, 0:n1 - n0], start=False, stop=True)
                    ot = proj.tile([128, c.VC], F32, tag="ot")
                    nc.vector.tensor_copy(ot[0:BT, 0:n1 - n0], po[0:BT, 0:n1 - n0])
                    nc.sync.dma_start(out_d[:, n0:n1], ot[0:BT, 0:n1 - n0])

    return nc


# ---------------------------------------------------------------------------
def _perm_cols(W3, NG, H):
    """[K, 3H] cols from (gate, h) to (group, gate, h-slice) order."""
    K = W3.shape[0]
    return np.ascontiguousarray(
        W3.reshape(K, 3, NG, H // NG).transpose(0, 2, 1, 3)).reshape(K, 3 * H)


def host_prep(inputs, c: Cfg):
    f32 = lambda x: np.asarray(x, np.float32)
    f16 = lambda x: np.ascontiguousarray(np.asarray(x, np.float32).astype(np.float16))
    H, E, T, TX, V, NG, B = c.H, c.E, c.T, c.TX, c.V, c.NG, c.B

    emb = f32(inputs["embedding"])
    x_t = np.asarray(inputs["x_t"]).astype(np.int64)[:, :T]
    va = f32(inputs["Va_w"])[0]
    shared = {
        "WaT": f16(f32(inputs["Wa_w"]).T),
        "UaT": f16(f32(inputs["Ua_w"]).T),
        "va": f16(va.reshape(c.KH, 128).T),
        "uab": np.ascontiguousarray(
            (f32(inputs["Ua_b"]) + f32(inputs["Wa_b"])).reshape(c.KH, 128).T
        ).astype(np.float32),
        "outwT": f16(f32(inputs["out_w"]).T[:, :V]),
        "outb": f16(f32(inputs["out_b"])[None, :V]),
        "ones16": np.ones((1, 256), np.float16),
        "ones32": np.ones((1, 128), np.float32),
    }
    for l in range(2):
        Wih = f32(inputs[f"gru{l}_Wih"]); Whh = f32(inputs[f"gru{l}_Whh"])
        bih = f32(inputs[f"gru{l}_bih"]); bhh = f32(inputs[f"gru{l}_bhh"])
        Din = Wih.shape[1] - 2 * H
        shared[f"WicT{l}"] = f16(_perm_cols(np.ascontiguousarray(Wih[:, Din:].T), NG, H))
        shared[f"WhhT{l}"] = f16(_perm_cols(np.ascontiguousarray(Whh.T), NG, H))
        gxbv = _perm_cols((bih + np.concatenate(
            [bhh[:2 * H], np.zeros(H, np.float32)]))[None, :], NG, H)[0]
        # block order (pc, kt): j = (kt//2)*6 + pc*2 + kt%2
        gxbT = np.zeros((128, 3 * c.KH), np.float32)
        for pcg in range(3):
            for kt in range(c.KH):
                j = (kt // 2) * 6 + pcg * 2 + (kt % 2)
                gxbT[:, pcg * c.KH + kt] = gxbv[j * 128:(j + 1) * 128]
        shared[f"gxb{l}"] = gxbT
        bn = bhh[2 * H:].reshape(c.KH, 128).T          # [128, KH]
        shared[f"bhhn{l}"] = np.ascontiguousarray(
            np.repeat(bn[:, :, None], B, axis=2).reshape(128, 2 * c.KH)
        ).astype(np.float32)
        shared[f"iW{l}"] = f16(f32(inputs["initialWs"])[l])
        W = f16(_perm_cols(np.ascontiguousarray(Wih[:, :Din].T), NG, H))
        shared["WixT0" if l == 0 else "WixT1"] = W

    ahe = f32(inputs["all_hidden_encoder"])
    in_maps = []
    for core in range(8):
        rows = [2 * core, 2 * core + 1]
        m = dict(shared)
        xe = emb[x_t[rows]]
        m["xT"] = f16(xe.transpose(2, 1, 0).reshape(E, B * T))
        for l in range(2):
            k = ahe[l, rows, :TX]
            m[f"keysT{l}"] = f16(k.transpose(2, 0, 1).reshape(2 * H, B * TX))
            m[f"keys{l}"] = f16(k.transpose(1, 0, 2).reshape(TX, B * 2 * H))
        in_maps.append(m)
    return in_maps


_NC_CACHE = {}


def kernel(**inputs) -> np.ndarray:
    c = FULL
    if "nc" not in _NC_CACHE:
        _NC_CACHE["nc"] = build_kernel(c)
    res = run_bass_kernel_spmd(_NC_CACHE["nc"], host_prep(inputs, c),
                               core_ids=list(range(8)))
    outs = []
    for core in range(8):
        o = res.results[core]["out"].reshape(c.T, c.B, c.V).transpose(1, 0, 2)
        outs.append(o)
    return np.concatenate(outs, axis=0).astype(np.float32)



# revision 8
# speedup vs baseline: 1.2283x; 1.2283x over previous
"""Trainium2 Bass kernel for nn_DecoderND_39058432590521.

Sharding: data-parallel over batch B=16 across 8 NeuronCores (B=2 rows per
core, zero collectives). Each core runs the full 2-layer additive-attention
GRU scan for its 2 batch rows.

Key optimizations over the naive scan:
 - context-GEMM factorization: gc = softmax_w @ (keys @ WicT); KWic is
   precomputed once per layer (dense M=128 matmuls), collapsing the per-step
   [2,2048]x[2048,3072] weight stream into a [2,128]x[128,3072] one.
 - attention adds fused into the scalar-engine activation (bias AP), scores
   matmul in (ht,b,tx) layout, softmax exp computed as tanh(x/8) -> exp(x)
   so the scalar engine only ever uses the Tanh table (no table swaps).
 - gate weight streams pre-scaled by SC=2048 (descaled at PSUM evacuation)
   so low-precision weight formats slot in without changing the scan.
 - gx kept SBUF-resident (freed by evicting Wic from the scan working set).
"""
import sys
sys.path.insert(0, '/opt/trn_rl_repo')
import numpy as np

import concourse.bass as bass
import concourse.mybir as mybir
import concourse.tile as tile
import bass_rust
from concourse.bass_utils import run_bass_kernel_spmd

F16 = mybir.dt.float16
F32 = mybir.dt.float32
AF = mybir.ActivationFunctionType
ALU = mybir.AluOpType

SC = 2048.0      # gate-stream scale (Whh/Wic pre-scaled; descale at evac)
RSC = 1.0 / SC


# ---------------------------------------------------------------------------
# This toolchain's walrus rejects >1 sync wait on TPB_CTRL instructions; the
# stock TileContext exit drain carries one wait per live processor. Split the
# waits one-per-nop ahead of a bare drain.
def _patched_drain_and_barrier(self, tick_clock, wait_clock):
    from concourse.tile import ScopedClock
    probe = self.nc.sync.nop(nofuse=True)
    wait_clock.add_sem_waits(probe.ins, ScopedClock({None: tick_clock.global_clock}))
    waits = list(probe.ins.sync_info.on_wait)
    probe.ins.sync_info = bass_rust.SyncInfo(on_wait=waits[:1], on_update=[])
    for w in waits[1:]:
        n = self.nc.sync.nop(nofuse=True)
        n.ins.sync_info = bass_rust.SyncInfo(on_wait=[w], on_update=[])
    self.nc.sync.drain()
    self.nc.all_engine_barrier()
    assert self.sems is not None
    popped = self.nc._tile_sem_poison_stack.pop()
    assert popped is self._sem_poison
    self.nc.clear_and_free_semaphores(list(self.sems.allocated().values()))
    self.nc.all_engine_barrier()


tile.TileContext._drain_and_barrier = _patched_drain_and_barrier


# Split any instruction carrying more than one sync wait: hoist the extra
# waits onto same-engine NOPs inserted immediately before it (this walrus
# build rejects multi-wait sync setup on several instruction classes).
def _split_excess_waits(nc, limit=1):
    def mknop(engine):
        eng = nc.engines[engine]
        inst = eng.nop(nofuse=True)
        for bb in nc.main_func.blocks:
            lst = bb.instructions
            if lst and lst[-1].name == inst.ins.name:
                bb.instructions = lst[:-1]
                break
        return inst.ins

    for bb in nc.main_func.blocks:
        changed = False
        out = []
        for inst in bb.instructions:
            si = inst.sync_info
            waits = list(si.on_wait) if si is not None else []
            if len(waits) > limit:
                for w in waits[:-limit]:
                    nop = mknop(inst.engine)
                    nop.sync_info = bass_rust.SyncInfo(on_wait=[w], on_update=[])
                    out.append(nop)
                inst.sync_info = bass_rust.SyncInfo(on_wait=waits[-limit:],
                                                    on_update=list(si.on_update))
                changed = True
            out.append(inst)
        if changed:
            bb.instructions = out


_orig_sched = tile.TileContext.schedule_and_allocate


def _patched_sched(self, *a, **k):
    r = _orig_sched(self, *a, **k)
    _split_excess_waits(self.nc)
    return r


tile.TileContext.schedule_and_allocate = _patched_sched


class Cfg:
    def __init__(self, T=64, V=32000, NG=4, debug_h=False):
        self.B = 2
        self.H, self.E, self.T, self.TX, self.V = 1024, 512, T, 128, V
        self.NG = NG
        self.KH = self.H // 128          # 8 k-tiles of H
        self.K2H = 2 * self.H // 128     # 16 k-tiles of 2H
        self.QW = self.H // NG           # 256
        self.GW = 3 * self.H // NG       # 768
        self.VC = 512
        self.debug_h = debug_h


FULL = Cfg()


def build_kernel(c: Cfg):
    nc = bass.Bass(target_bir_lowering=False)
    B, H, E, T, TX, V, NG = c.B, c.H, c.E, c.T, c.TX, c.V, c.NG
    KH, K2H, QW, GW = c.KH, c.K2H, c.QW, c.GW
    H3, BT, KB = 3 * H, B * T, 2 * c.KH
    assert B == 2 and TX == 128

    def dram_in(name, shape, dt=F16):
        return nc.dram_tensor(name, shape, dt, kind="ExternalInput")

    xT_d = dram_in("xT", [E, BT])
    WaT_d = dram_in("WaT", [H, H])
    UaT_d = dram_in("UaT", [2 * H, H])
    va_d = dram_in("va", [128, KH])
    ones16_d = dram_in("ones16", [1, 256])
    uab_d = dram_in("uab", [128, H // 128], F32)
    WixT0_d = dram_in("WixT0", [E, H3])
    WixT1_d = dram_in("WixT1", [H, H3])
    WicT_d = [dram_in(f"WicT{l}", [2 * H, H3]) for l in range(2)]   # pre-scaled SC
    WhhT_d = [dram_in(f"WhhT{l}", [H, H3]) for l in range(2)]       # pre-scaled SC
    gxb_d = [dram_in(f"gxb{l}", [128, 3 * H // 128], F32) for l in range(2)]
    bhhn_d = [dram_in(f"bhhn{l}", [128, 2 * H // 128], F32) for l in range(2)]
    keysT_d = [dram_in(f"keysT{l}", [2 * H, B * TX]) for l in range(2)]
    iW_d = [dram_in(f"iW{l}", [H, H]) for l in range(2)]
    outwT_d = dram_in("outwT", [H, V])
    outb_d = dram_in("outb", [1, V])

    out_d = nc.dram_tensor("out", [BT, V], F32, kind="ExternalOutput")
    if c.debug_h:
        hdbg = [nc.dram_tensor(f"hdbg{l}", [128, T * KB], F16,
                               kind="ExternalOutput") for l in range(2)]
        dbg = {}
        for nm, shp, dt in [("dq", [128, KB], F32), ("dA", [128, 8 * B * 128], F16),
                            ("dw", [1, B * 128], F32), ("dZ", [1, B], F32),
                            ("dg", [128, 4 * KB], F32),
                            ("dgx", [128, 3 * KB], F32),
                            ("dgab", [128, GW + QW], F16),
                            ("dkwic", [128, B * 64], F16),
                            ("dpsc", [1, B * 128], F32)]:
            dbg[nm] = nc.dram_tensor(nm, shp, dt, kind="ExternalOutput")

    def r_kt(d, inner=128):
        return d.ap().rearrange("(kt k) n -> k kt n", k=inner)

    with tile.TileContext(nc) as tc:
        import contextlib
        with contextlib.ExitStack() as ctx:
            wpool = ctx.enter_context(tc.tile_pool(name="wsmall", bufs=1))
            spool = ctx.enter_context(tc.tile_pool(name="state", bufs=1))

            va_sb = wpool.tile([128, KH], F16)
            ones16 = wpool.tile([1, 256], F16)
            id1 = wpool.tile([1, 1], F16)
            bhhn = wpool.tile([128, KB], F32)

            UaK = spool.tile([128, KH, B * 128], F16)
            KWic = [spool.tile([128, B, H3], F16, tag=f"KWic{l}", name=f"KWic{l}")
                    for l in range(2)]
            gx_sb = [spool.tile([128, 3 * KH, BT], F16, tag=f"gx{l}", name=f"gx{l}")
                     for l in range(2)]
            hsT = [spool.tile([128, KH, T, B], F16, tag=f"hsT{l}", name=f"hsT{l}")
                   for l in range(2)]
            h32 = spool.tile([128, KB], F32)
            h16i = spool.tile([128, KB], F16)
            A16 = spool.tile([128, KH, B * 128], F16)
            q_sb = spool.tile([128, QW], F16)
            qT32 = spool.tile([128, KB], F32)
            trow = spool.tile([1, B, 128], F32)
            e1row = spool.tile([1, B, 128], F32)
            e2row = spool.tile([1, B, 128], F32)
            w2row = spool.tile([1, B, 128], F16)
            Zrow = spool.tile([1, B], F32)
            rZrow = spool.tile([1, B], F32)
            # block-diagonal softmax weights: wTpad[:, b, b] = w_b, rest 0
            wTpad = spool.tile([128, B, B], F16)
            gAB_sb = spool.tile([128, GW + QW], F16)
            g48f = spool.tile([128, 4 * KB], F32)
            gxt = spool.tile([128, 3 * KB], F32)
            id128 = spool.tile([128, 128], F16)
            rzp = spool.tile([128, 2 * KB], F32)
            rzt = spool.tile([128, 2 * KB], F32)
            hn = spool.tile([128, KB], F32)
            nin = spool.tile([128, KB], F32)
            ngate = spool.tile([128, KB], F32)
            tmpg = spool.tile([128, KB], F32)

            from concourse.masks import make_identity
            nc.gpsimd.memset(ones16[:], 1.0)
            nc.gpsimd.memset(id1[:], 1.0)
            nc.gpsimd.memset(wTpad[:], 0.0)
            make_identity(nc, id128[:])
            nc.sync.dma_start(va_sb[:], va_d[:])
            nc.sync.dma_start(bhhn[:], bhhn_d[0][:])

            # ---------------- per-layer prep ----------------
            def prep_layer(l, pp, pspool):
                # UaK = Ua @ keys^T (+ combined bias), per h-tile
                UaT_sb = pp.tile([128, K2H, H], F16, tag="UaT")
                keysT_sb = pp.tile([128, K2H, B * TX], F16, tag="keysT")
                iW_sb = pp.tile([128, KH, H], F16, tag="iW")
                uab_sb = pp.tile([128, KH], F32, tag="uab")
                nc.sync.dma_start(UaT_sb[:], r_kt(UaT_d))
                nc.sync.dma_start(keysT_sb[:], r_kt(keysT_d[l]))
                nc.sync.dma_start(iW_sb[:], r_kt(iW_d[l]))
                nc.sync.dma_start(uab_sb[:], uab_d[:])
                for ht in range(KH):
                    pu = pspool.tile([128, B * TX], F32, tag="pu")
                    for kt in range(K2H):
                        nc.tensor.matmul(pu[:], UaT_sb[:, kt, ht * 128:(ht + 1) * 128],
                                         keysT_sb[:, kt, :], start=(kt == 0),
                                         stop=(kt == K2H - 1))
                    nc.vector.tensor_scalar_add(UaK[:, ht, :], pu[:],
                                                uab_sb[:, ht:ht + 1])
                # s0 = keys[:,0,H:] @ iW  -> h32 (transposed layout)
                for ht in range(KH):
                    ps0 = pspool.tile([128, B], F32, tag="ps0")
                    for kt in range(KH):
                        rhs = keysT_sb[:, KH + kt, :].rearrange(
                            "k (b t) -> k b t", b=B)[:, :, 0]
                        nc.tensor.matmul(ps0[:], iW_sb[:, kt, ht * 128:(ht + 1) * 128],
                                         rhs, start=(kt == 0), stop=(kt == KH - 1))
                    nc.vector.tensor_copy(h32[:, ht * 2:(ht + 1) * 2], ps0[:])
                # KWic[b] = keys[b] @ WicT(pre-scaled): lhsT=keysT slice [128,TX].
                # Wic streamed in 2 column-halves to bound SBUF.
                HW2 = H3 // 2
                for half in range(2):
                    wic_sb = pp.tile([128, K2H, HW2], F16, tag="wic")
                    nc.sync.dma_start(
                        wic_sb[:], r_kt(WicT_d[l])[:, :, half * HW2:(half + 1) * HW2])
                    for b in range(B):
                        for nchunk in range(HW2 // 512):
                            col0 = nchunk * 512
                            pk = pspool.tile([128, 512], F32, tag="pkwic")
                            for kt in range(K2H):
                                nc.tensor.matmul(
                                    pk[:],
                                    keysT_sb[:, kt, b * TX:(b + 1) * TX],
                                    wic_sb[:, kt, col0:col0 + 512],
                                    start=(kt == 0), stop=(kt == K2H - 1))
                            nc.vector.tensor_copy(
                                KWic[l][:, b, half * HW2 + col0:half * HW2 + col0 + 512],
                                pk[:])

            def gx_compute(l, rhsT, KD, WixT_t, pp, pspool):
                # transposed: gx block (pc, kt) = WixT-cols.T @ xT  [128, BT]
                gxb_sb = pp.tile([128, 3 * KH], F32, tag="gxb")
                nc.sync.dma_start(gxb_sb[:], gxb_d[l][:])
                for pcg in range(3):
                    for kt in range(KH):
                        j = (kt // 2) * 6 + pcg * 2 + (kt % 2)
                        pgx = pspool.tile([128, BT], F32, tag="pgx")
                        for kd in range(KD):
                            nc.tensor.matmul(pgx[:], WixT_t[:, kd, j * 128:(j + 1) * 128],
                                             rhsT(kd), start=(kd == 0),
                                             stop=(kd == KD - 1))
                        blk = pcg * KH + kt
                        nc.vector.tensor_scalar_add(gx_sb[l][:, blk, :], pgx[:],
                                                    gxb_sb[:, blk:blk + 1])

            # ---------------- the scan ----------------
            def scan_layer(l, WaT, WhhT, ps):
                pq = ps.tile([128, QW], F32, tag="pq", name=f"pq{l}")
                pg = ps.tile([128, GW + QW], F32, tag="pg", name=f"pg{l}")
                psc = ps.tile([128, B * 128], F32, tag="psc", name=f"psc{l}")
                ptr = ps.tile([128, 8, 128], F16, tag="ptr", name=f"ptr{l}")
                # dummy-init full tiles so evacuation reads see owned data
                for nnn in range(0, QW, 256):
                    nc.tensor.matmul(pq[:, nnn:nnn + 256], ones16[0:1, 0:128],
                                     ones16[0:1, 0:256], start=True, stop=True)
                for nnn in range(0, GW + QW, 256):
                    nc.tensor.matmul(pg[:, nnn:nnn + 256], ones16[0:1, 0:128],
                                     ones16[0:1, 0:256], start=True, stop=True)
                nc.vector.tensor_copy(h16i[:], h32[:])
                gxv = gx_sb[l][:].rearrange("p blk tb -> p (blk tb)")
                for t in range(T):
                    def hsl(kt, _t=t):
                        if _t == 0:
                            return h16i[:, kt * 2:kt * 2 + 2]
                        return hsT[l][:, kt, _t - 1, :]
                    # q (batch-stationary, col-tiled, 4 PE column groups)
                    for kt in range(KH):
                        for g in range(NG):
                            nc.tensor.matmul(
                                pq[32 * g:32 * g + 2, :], hsl(kt),
                                WaT[:, kt, g * QW:(g + 1) * QW],
                                start=(kt == 0), stop=(kt == KH - 1),
                                tile_position=(0, 32 * g), skip_group_check=True)
                    # gh into gates psum: rz -> [0:2QW], ghn -> [GW:GW+QW]
                    for kt in range(KH):
                        for g in range(NG):
                            nc.tensor.matmul(
                                pg[32 * g:32 * g + 2, 0:2 * QW],
                                hsl(kt),
                                WhhT[:, kt, g * GW:g * GW + 2 * QW],
                                start=(kt == 0), stop=False,
                                tile_position=(0, 32 * g), skip_group_check=True)
                            nc.tensor.matmul(
                                pg[32 * g:32 * g + 2, GW:GW + QW],
                                hsl(kt),
                                WhhT[:, kt, g * GW + 2 * QW:(g + 1) * GW],
                                start=(kt == 0), stop=(kt == KH - 1),
                                tile_position=(0, 32 * g), skip_group_check=True)
                    # qT: evac + PE transpose + strided gather
                    nc.scalar.copy(q_sb[:], pq[:])
                    for kl in range(2):
                        nc.tensor.transpose(ptr[:, kl, :],
                                            q_sb[:, kl * 128:(kl + 1) * 128],
                                            id128[:])
                    # qT32[p, (2g+kl)*2+b] = ptr[p, kl, 32g+b]
                    gsrc = ptr[:, 0:2, :].rearrange("p kl (g b) -> p kl g b", b=32)[
                        :, :, :, 0:2]
                    gdst = qT32[:].rearrange("p (g kl b) -> p kl g b", kl=2, g=NG)
                    nc.vector.tensor_copy(gdst, gsrc)
                    # A = tanh(UaK + qT) fused via bias AP, per (ht, b)
                    for ht in range(KH):
                        for b in range(B):
                            nc.scalar.activation(
                                A16[:, ht, b * 128:(b + 1) * 128],
                                UaK[:, ht, b * 128:(b + 1) * 128],
                                AF.Tanh, bias=qT32[:, ht * 2 + b:ht * 2 + b + 1])
                    # scores: accumulate va.T @ A over h-tiles -> psc [1, B*128]
                    for ht in range(KH):
                        nc.tensor.matmul(
                            psc[0:1, :], va_sb[:, ht:ht + 1], A16[:, ht, :],
                            start=(ht == 0), stop=(ht == KH - 1),
                            skip_group_check=True)
                    # softmax via tanh-only exp: exp(x) = ((1+t)/(1-t))^4, t=tanh(x/8)
                    nc.scalar.activation(trow[0:1, :, :].rearrange("o b x -> o (b x)"),
                                         psc[0:1, :], AF.Tanh, scale=0.125)
                    tr = trow[0:1, :, :].rearrange("o b x -> o (b x)")
                    e1 = e1row[0:1, :, :].rearrange("o b x -> o (b x)")
                    e2 = e2row[0:1, :, :].rearrange("o b x -> o (b x)")
                    nc.vector.tensor_scalar_sub(e2[:], tr[:], 1.0)      # t-1
                    nc.vector.reciprocal(e1[:], e2[:])                  # 1/(t-1)
                    nc.vector.tensor_scalar_add(e2[:], tr[:], 1.0)      # 1+t
                    nc.vector.tensor_mul(e1[:], e1[:], e2[:])           # -exp(x/4)
                    nc.vector.tensor_mul(e2[:], e1[:], e1[:])           # exp(x/2)
                    for b in range(B):
                        nc.vector.scalar_tensor_tensor(
                            e1row[0:1, b, :], e2row[0:1, b, :], 1.0,
                            e2row[0:1, b, :], ALU.mult, ALU.mult,
                            accum_out=Zrow[0:1, b:b + 1])               # exp(x), Z
                    nc.vector.reciprocal(rZrow[:], Zrow[:])
                    for b in range(B):
                        nc.vector.tensor_scalar_mul(w2row[0:1, b, :],
                                                    e1row[0:1, b, :],
                                                    rZrow[0:1, b:b + 1])
                    for b in range(B):
                        nc.tensor.transpose(ptr[:, 7, 2 * b:2 * b + 1],
                                            w2row[0:1, b, :], id1[:])
                    # scatter w_b onto the block diagonal of wTpad
                    nc.vector.tensor_copy(
                        wTpad[:].rearrange("p a b -> p (a b)")[:, 0:B * B:B + 1],
                        ptr[:, 7, 0:4].rearrange(
                            "p (b o) -> p b o", o=2)[:, :, 0])
                    # gkc: out[b,:] = w_b @ KWic[b] via block-diag lhsT over
                    # K=2*TX (k-tile kt multiplies KWic[b=kt]); M=2 aligned.
                    for kt in range(B):
                        for g in range(NG):
                            nc.tensor.matmul(
                                pg[32 * g:32 * g + 2, 0:2 * QW],
                                wTpad[:, kt, :],
                                KWic[l][:, kt, g * GW:g * GW + 2 * QW],
                                start=False, stop=(kt == B - 1),
                                tile_position=(0, 32 * g), skip_group_check=True)
                            nc.tensor.matmul(
                                pg[32 * g:32 * g + 2, 2 * QW:3 * QW],
                                wTpad[:, kt, :],
                                KWic[l][:, kt, g * GW + 2 * QW:(g + 1) * GW],
                                start=(kt == 0), stop=(kt == B - 1),
                                tile_position=(0, 32 * g), skip_group_check=True)
                    # gates: evac (descale) + PE transposes + strided gathers
                    nc.vector.tensor_scalar_mul(gAB_sb[:], pg[:], RSC)
                    for j in range(8):
                        nc.tensor.transpose(ptr[:, j, :],
                                            gAB_sb[:, j * 128:(j + 1) * 128],
                                            id128[:])
                    # g48f[p, pc*16+(2g+kl)*2+b] = ptr[p, pc*2+kl, 32g+b]
                    for kl in range(2):
                        gsrc2 = ptr[:, :, :].rearrange(
                            "p (pc kl) (g b) -> p kl pc g b", kl=2, b=32)[
                            :, kl, :, :, 0:2]
                        gdst2 = g48f[:].rearrange(
                            "p (pc g kl b) -> p kl pc g b", pc=4, g=NG, kl=2)[:, kl]
                        nc.vector.tensor_copy(gdst2, gsrc2)
                    # gx slice for this t (SBUF-resident)
                    nc.vector.tensor_copy(
                        gxt[:].rearrange("p (blk b) -> p blk b", b=B),
                        gx_sb[l][:, :, 2 * t:2 * t + 2])
                    if c.debug_h and t == 0 and l == 0:
                        nc.sync.dma_start(dbg["dq"][:], qT32[:])
                        nc.sync.dma_start(dbg["dA"][:],
                                          A16[:].rearrange("p k x -> p (k x)"))
                        nc.sync.dma_start(dbg["dpsc"][:], psc[0:1, :])
                        nc.sync.dma_start(dbg["dZ"][:], Zrow[:])
                        nc.sync.dma_start(
                            dbg["dw"][:],
                            e1row[0:1, :, :].rearrange("o b x -> o (b x)"))
                        nc.sync.dma_start(dbg["dg"][:], g48f[:])
                        nc.sync.dma_start(dbg["dgx"][:], gxt[:])
                        nc.sync.dma_start(dbg["dgab"][:], gAB_sb[:])
                        nc.sync.dma_start(
                            dbg["dkwic"][:],
                            KWic[l][:, :, 0:64].rearrange("p b x -> p (b x)"))
                    # gates elementwise (fp32), tanh-only activations:
                    # r,z = sigmoid(x) = 0.5*tanh(x/2)+0.5
                    nc.vector.tensor_add(rzp[:], g48f[:, 0:2 * KB], gxt[:, 0:2 * KB])
                    nc.scalar.activation(rzt[:], rzp[:], AF.Tanh, scale=0.5)
                    nc.vector.tensor_add(hn[:], g48f[:, 3 * KB:4 * KB], bhhn[:])
                    # nin = gx_n + gc_n + 0.5*(1+t_r)*hn
                    nc.vector.scalar_tensor_tensor(
                        tmpg[:], rzt[:, 0:KB], 1.0, hn[:], ALU.add, ALU.mult)
                    nc.vector.tensor_add(nin[:], g48f[:, 2 * KB:3 * KB],
                                         gxt[:, 2 * KB:3 * KB])
                    nc.vector.scalar_tensor_tensor(
                        nin[:], tmpg[:], 0.5, nin[:], ALU.mult, ALU.add)
                    nc.scalar.activation(ngate[:], nin[:], AF.Tanh)
                    # h = n + 0.5*(1+t_z)*(h - n)
                    nc.vector.tensor_sub(tmpg[:], h32[:], ngate[:])
                    nc.vector.scalar_tensor_tensor(
                        tmpg[:], rzt[:, KB:2 * KB], 1.0, tmpg[:], ALU.add, ALU.mult)
                    nc.vector.scalar_tensor_tensor(
                        h32[:], tmpg[:], 0.5, ngate[:], ALU.mult, ALU.add)
                    nc.vector.tensor_copy(
                        hsT[l][:, :, t, :],
                        h32[:].rearrange("p (kt b) -> p kt b", b=B))
                if c.debug_h:
                    nc.sync.dma_start(
                        hdbg[l][:],
                        hsT[l][:, :, :, :].rearrange("p kt t b -> p (kt t b)"))

            # ================= phases =================
            with tc.tile_pool(name="prep0", bufs=1) as pp, \
                 tc.tile_pool(name="psA", bufs=1, space="PSUM") as psA:
                prep_layer(0, pp, psA)
            with tc.tile_pool(name="gxp0", bufs=1) as pp, \
                 tc.tile_pool(name="psA2", bufs=1, space="PSUM") as psA2:
                WixT0_sb = pp.tile([128, E // 128, H3], F16, tag="Wix")
                xT_sb = pp.tile([128, E // 128, BT], F16, tag="xTs")
                nc.sync.dma_start(WixT0_sb[:], r_kt(WixT0_d))
                nc.sync.dma_start(xT_sb[:], r_kt(xT_d))
                gx_compute(0, lambda kt: xT_sb[:, kt, :], E // 128, WixT0_sb, pp, psA2)

            for l in range(2):
                if l == 1:
                    nc.sync.dma_start(bhhn[:], bhhn_d[1][:])
                    with tc.tile_pool(name="prep1", bufs=1) as pp, \
                         tc.tile_pool(name="psB", bufs=1, space="PSUM") as psB:
                        prep_layer(1, pp, psB)
                    with tc.tile_pool(name="gxp1", bufs=1) as pp, \
                         tc.tile_pool(name="psB2", bufs=1, space="PSUM") as psB2:
                        WixT1_sb = pp.tile([128, KH, H3], F16, tag="Wix1")
                        nc.sync.dma_start(WixT1_sb[:], r_kt(WixT1_d))
                        gx_compute(1, lambda kt: hsT[0][:, kt, :, :].rearrange(
                                       "p t b -> p (t b)"),
                                   KH, WixT1_sb, pp, psB2)
                with tc.tile_pool(name=f"bigw{l}", bufs=1) as bw, \
                     tc.tile_pool(name=f"psS{l}", bufs=1, space="PSUM") as ps:
                    WaT = bw.tile([128, KH, H], F16, tag="WaT")
                    WhhT = bw.tile([128, KH, H3], F16, tag="WhhT")
                    nc.sync.dma_start(WaT[:], r_kt(WaT_d))
                    nc.sync.dma_start(WhhT[:], r_kt(WhhT_d[l]))
                    scan_layer(l, WaT, WhhT, ps)

            # ---- output projection ----
            with tc.tile_pool(name="proj", bufs=3) as proj, \
                 tc.tile_pool(name="psP", bufs=2, space="PSUM") as psP:
                skipT = spool.tile([128, T * KB], F16, tag="skipT")
                nc.vector.tensor_add(
                    skipT[:],
                    hsT[0][:, :, :, :].rearrange("p kt t b -> p (kt t b)"),
                    hsT[1][:, :, :, :].rearrange("p kt t b -> p (kt t b)"))
                sk3 = skipT[:].rearrange("p (kt tb) -> p kt tb", kt=KH)
                NCH = (V + c.VC - 1) // c.VC
                for nci in range(NCH):
                    n0 = nci * c.VC
                    n1 = min(V, n0 + c.VC)
                    wchunk = proj.tile([128, KH, c.VC], F16, tag="wchunk")
                    nc.sync.dma_start(wchunk[:, :, 0:n1 - n0],
                                      r_kt(outwT_d)[:, :, n0:n1])
                    obc = proj.tile([1, c.VC], F16, tag="obc")
                    nc.sync.dma_start(obc[0:1, 0:n1 - n0], outb_d[0:1, n0:n1])
                    po = psP.tile([128, c.VC], F32, tag="pout")
                    for kt in range(KH):
                        nc.tensor.matmul(po[0:BT, 0:n1 - n0],
                                         sk3[:, kt, :],
                                         wchunk[:, kt, 0:n1 - n0],
                                         start=(kt == 0), stop=False)
                    nc.tensor.matmul(po[0:BT, 0:n1 - n0], ones16[0:1, 0:BT],
                                     obc[0:1, 0:n1 - n0], start=False, stop=True)
                    ot = proj.tile([128, c.VC], F32, tag="ot")
                    nc.vector.tensor_copy(ot[0:BT, 0:n1 - n0], po[0:BT, 0:n1 - n0])
                    nc.sync.dma_start(out_d[:, n0:n1], ot[0:BT, 0:n1 - n0])

    return nc


# ---------------------------------------------------------------------------
def _perm_cols(W3, NG, H):
    """[K, 3H] cols from (gate, h) to (group, gate, h-slice) order."""
    K = W3.shape[0]
    return np.ascontiguousarray(
        W3.reshape(K, 3, NG, H // NG).transpose(0, 2, 1, 3)).reshape(K, 3 * H)


def host_prep(inputs, c: Cfg):
    f32 = lambda x: np.asarray(x, np.float32)
    f16 = lambda x: np.ascontiguousarray(np.asarray(x, np.float32).astype(np.float16))
    H, E, T, TX, V, NG, B = c.H, c.E, c.T, c.TX, c.V, c.NG, c.B

    emb = f32(inputs["embedding"])
    x_t = np.asarray(inputs["x_t"]).astype(np.int64)[:, :T]
    va = f32(inputs["Va_w"])[0]
    shared = {
        "WaT": f16(f32(inputs["Wa_w"]).T),
        "UaT": f16(f32(inputs["Ua_w"]).T),
        "va": f16(va.reshape(c.KH, 128).T),
        "uab": np.ascontiguousarray(
            (f32(inputs["Ua_b"]) + f32(inputs["Wa_b"])).reshape(c.KH, 128).T
        ).astype(np.float32),
        "outwT": f16(f32(inputs["out_w"]).T[:, :V]),
        "outb": f16(f32(inputs["out_b"])[None, :V]),
        "ones16": np.ones((1, 256), np.float16),
    }
    for l in range(2):
        Wih = f32(inputs[f"gru{l}_Wih"]); Whh = f32(inputs[f"gru{l}_Whh"])
        bih = f32(inputs[f"gru{l}_bih"]); bhh = f32(inputs[f"gru{l}_bhh"])
        Din = Wih.shape[1] - 2 * H
        shared[f"WicT{l}"] = f16(
            _perm_cols(np.ascontiguousarray(Wih[:, Din:].T), NG, H) * SC)
        shared[f"WhhT{l}"] = f16(
            _perm_cols(np.ascontiguousarray(Whh.T), NG, H) * SC)
        gxbv = _perm_cols((bih + np.concatenate(
            [bhh[:2 * H], np.zeros(H, np.float32)]))[None, :], NG, H)[0]
        # block order (pc, kt): j = (kt//2)*6 + pc*2 + kt%2
        gxbT = np.zeros((128, 3 * c.KH), np.float32)
        for pcg in range(3):
            for kt in range(c.KH):
                j = (kt // 2) * 6 + pcg * 2 + (kt % 2)
                gxbT[:, pcg * c.KH + kt] = gxbv[j * 128:(j + 1) * 128]
        shared[f"gxb{l}"] = gxbT
        bn = bhh[2 * H:].reshape(c.KH, 128).T          # [128, KH]
        shared[f"bhhn{l}"] = np.ascontiguousarray(
            np.repeat(bn[:, :, None], B, axis=2).reshape(128, 2 * c.KH)
        ).astype(np.float32)
        shared[f"iW{l}"] = f16(f32(inputs["initialWs"])[l])
        W = f16(_perm_cols(np.ascontiguousarray(Wih[:, :Din].T), NG, H))
        shared["WixT0" if l == 0 else "WixT1"] = W

    ahe = f32(inputs["all_hidden_encoder"])
    in_maps = []
    for core in range(8):
        rows = [2 * core, 2 * core + 1]
        m = dict(shared)
        xe = emb[x_t[rows]]
        m["xT"] = f16(xe.transpose(2, 1, 0).reshape(E, B * T))
        for l in range(2):
            k = ahe[l, rows, :TX]
            m[f"keysT{l}"] = f16(k.transpose(2, 0, 1).reshape(2 * H, B * TX))
        in_maps.append(m)
    return in_maps


_NC_CACHE = {}


def kernel(**inputs) -> np.ndarray:
    c = FULL
    if "nc" not in _NC_CACHE:
        _NC_CACHE["nc"] = build_kernel(c)
    res = run_bass_kernel_spmd(_NC_CACHE["nc"], host_prep(inputs, c),
                               core_ids=list(range(8)))
    outs = []
    for core in range(8):
        o = res.results[core]["out"].reshape(c.T, c.B, c.V).transpose(1, 0, 2)
        outs.append(o)
    return np.concatenate(outs, axis=0).astype(np.float32)


# revision 23
# speedup vs baseline: 1.3770x; 1.1211x over previous
"""Trainium2 Bass kernel for nn_DecoderND_39058432590521.

Sharding: data-parallel over batch B=16 across 8 NeuronCores (B=2 rows per
core, zero collectives). Each core runs the full 2-layer additive-attention
GRU scan for its 2 batch rows.

Key optimizations over the naive scan:
 - context-GEMM factorization: gc = softmax_w @ (keys @ WicT); KWic is
   precomputed once per layer (dense M=128 matmuls), collapsing the per-step
   [2,2048]x[2048,3072] weight stream into a [2,128]x[128,3072] one.
 - attention adds fused into the scalar-engine activation (bias AP), scores
   matmul in (ht,b,tx) layout, softmax exp computed as tanh(x/8) -> exp(x)
   so the scalar engine only ever uses the Tanh table (no table swaps).
 - gate weight streams pre-scaled by SC=2048 (descaled at PSUM evacuation)
   so low-precision weight formats slot in without changing the scan.
 - gx kept SBUF-resident (freed by evicting Wic from the scan working set).
"""
import sys
sys.path.insert(0, '/opt/trn_rl_repo')
import numpy as np

import concourse.bass as bass
import concourse.mybir as mybir
import concourse.tile as tile
import bass_rust
from concourse.bass_utils import run_bass_kernel_spmd

F16 = mybir.dt.float16
F32 = mybir.dt.float32
F8 = mybir.dt.float8e4
AF = mybir.ActivationFunctionType
ALU = mybir.AluOpType
DR = mybir.MatmulPerfMode.DoubleRow

FP8 = False      # fp8 DoubleRow needs full 128-wide PE tiles; unusable here
SC = 2048.0      # gate-stream scale (Whh/Wic pre-scaled; descale at evac)
RSC = 1.0 / SC
HS = 16.0        # h fp8 scale;  weight fp8 scale = SC/HS = 128
RHS_ = 1.0 / HS


# ---------------------------------------------------------------------------
# This toolchain's walrus rejects >1 sync wait on TPB_CTRL instructions; the
# stock TileContext exit drain carries one wait per live processor. Split the
# waits one-per-nop ahead of a bare drain.
def _patched_drain_and_barrier(self, tick_clock, wait_clock):
    from concourse.tile import ScopedClock
    probe = self.nc.sync.nop(nofuse=True)
    wait_clock.add_sem_waits(probe.ins, ScopedClock({None: tick_clock.global_clock}))
    waits = list(probe.ins.sync_info.on_wait)
    probe.ins.sync_info = bass_rust.SyncInfo(on_wait=waits[:1], on_update=[])
    for w in waits[1:]:
        n = self.nc.sync.nop(nofuse=True)
        n.ins.sync_info = bass_rust.SyncInfo(on_wait=[w], on_update=[])
    self.nc.sync.drain()
    self.nc.all_engine_barrier()
    assert self.sems is not None
    popped = self.nc._tile_sem_poison_stack.pop()
    assert popped is self._sem_poison
    self.nc.clear_and_free_semaphores(list(self.sems.allocated().values()))
    self.nc.all_engine_barrier()


tile.TileContext._drain_and_barrier = _patched_drain_and_barrier


# Split any instruction carrying more than one sync wait: hoist the extra
# waits onto same-engine NOPs inserted immediately before it (this walrus
# build rejects multi-wait sync setup on several instruction classes).
def _split_excess_waits(nc, limit=1):
    def mknop(engine):
        eng = nc.engines[engine]
        inst = eng.nop(nofuse=True)
        for bb in nc.main_func.blocks:
            lst = bb.instructions
            if lst and lst[-1].name == inst.ins.name:
                bb.instructions = lst[:-1]
                break
        return inst.ins

    for bb in nc.main_func.blocks:
        changed = False
        out = []
        for inst in bb.instructions:
            si = inst.sync_info
            waits = list(si.on_wait) if si is not None else []
            if len(waits) > limit:
                for w in waits[:-limit]:
                    nop = mknop(inst.engine)
                    nop.sync_info = bass_rust.SyncInfo(on_wait=[w], on_update=[])
                    out.append(nop)
                inst.sync_info = bass_rust.SyncInfo(on_wait=waits[-limit:],
                                                    on_update=list(si.on_update))
                changed = True
            out.append(inst)
        if changed:
            bb.instructions = out


_orig_sched = tile.TileContext.schedule_and_allocate


def _patched_sched(self, *a, **k):
    r = _orig_sched(self, *a, **k)
    _split_excess_waits(self.nc)
    return r


tile.TileContext.schedule_and_allocate = _patched_sched


class Cfg:
    def __init__(self, T=64, V=32000, NG=4, debug_h=False):
        self.B = 2
        self.H, self.E, self.T, self.TX, self.V = 1024, 512, T, 128, V
        self.NG = NG
        self.KH = self.H // 128          # 8 k-tiles of H
        self.K2H = 2 * self.H // 128     # 16 k-tiles of 2H
        self.QW = self.H // NG           # 256
        self.GW = 3 * self.H // NG       # 768
        self.VC = 512
        self.debug_h = debug_h


FULL = Cfg()


def build_kernel(c: Cfg):
    nc = bass.Bass(target_bir_lowering=False)
    B, H, E, T, TX, V, NG = c.B, c.H, c.E, c.T, c.TX, c.V, c.NG
    KH, K2H, QW, GW = c.KH, c.K2H, c.QW, c.GW
    H3, BT, KB = 3 * H, B * T, 2 * c.KH
    assert B == 2 and TX == 128

    def dram_in(name, shape, dt=F16):
        return nc.dram_tensor(name, shape, dt, kind="ExternalInput")

    xT_d = dram_in("xT", [E, BT])
    WaT_d = dram_in("WaT", [H, H], F8 if FP8 else F16)
    UaT_d = dram_in("UaT", [2 * H, H])
    va_d = dram_in("va", [128, KH])
    ones16_d = dram_in("ones16", [1, 256])
    uab_d = dram_in("uab", [128, H // 128], F32)
    WixT0_d = dram_in("WixT0", [E, H3])
    WixT1_d = dram_in("WixT1", [H, H3])
    WicT_d = [dram_in(f"WicT{l}", [2 * H, H3]) for l in range(2)]   # pre-scaled SC
    WhhT_d = [dram_in(f"WhhT{l}", [H, H3], F8 if FP8 else F16)
              for l in range(2)]                                    # pre-scaled
    gxb_d = [dram_in(f"gxb{l}", [128, 3 * H // 128], F32) for l in range(2)]
    bhhn_d = [dram_in(f"bhhn{l}", [128, 2 * H // 128], F32) for l in range(2)]
    keysT_d = [dram_in(f"keysT{l}", [2 * H, B * TX]) for l in range(2)]
    iW_d = [dram_in(f"iW{l}", [H, H]) for l in range(2)]
    outwT_d = dram_in("outwT", [H, V])
    outb_d = dram_in("outb", [1, V])

    out_d = nc.dram_tensor("out", [BT, V], F32, kind="ExternalOutput")
    if c.debug_h:
        hdbg = [nc.dram_tensor(f"hdbg{l}", [128, T * KB], F16,
                               kind="ExternalOutput") for l in range(2)]
        dbg = {}
        for nm, shp, dt in [("dq", [128, KB], F32), ("dA", [128, 8 * B * 128], F16),
                            ("dw", [128, 4], F16),
                            ("dg", [128, 4 * KB], F32),
                            ("dgx", [128, 3 * KB], F32),
                            ("dgab", [128, GW + QW], F16),
                            ("dkwic", [128, B * 64], F16),
                            ("dpsc", [1, B * 128], F32)]:
            dbg[nm] = nc.dram_tensor(nm, shp, dt, kind="ExternalOutput")

    def r_kt(d, inner=128):
        return d.ap().rearrange("(kt k) n -> k kt n", k=inner)

    with tile.TileContext(nc) as tc:
        import contextlib
        with contextlib.ExitStack() as ctx:
            wpool = ctx.enter_context(tc.tile_pool(name="wsmall", bufs=1))
            spool = ctx.enter_context(tc.tile_pool(name="state", bufs=1))

            va_sb = wpool.tile([128, KH], F16)
            ones16 = wpool.tile([1, 256], F16)
            ones128 = wpool.tile([128, 1], F16)
            id1 = wpool.tile([1, 1], F16)
            id1f = wpool.tile([1, 1], F32)
            bhhn = wpool.tile([128, KB], F32)

            UaK = spool.tile([128, KH, B * 128], F16)
            KWic = [spool.tile([128, B, H3], F16, tag=f"KWic{l}", name=f"KWic{l}")
                    for l in range(2)]
            gx_sb = [spool.tile([128, 3 * KH, BT], F16, tag=f"gx{l}", name=f"gx{l}")
                     for l in range(2)]
            hsT = [spool.tile([128, KH, T, B], F16, tag=f"hsT{l}", name=f"hsT{l}")
                   for l in range(2)]
            h32 = spool.tile([128, KB], F32)
            h16i = spool.tile([128, KB], F16)
            h8 = spool.tile([128, KH, B], F8)
            A16 = spool.tile([128, KH, B * 128], F16)
            q_sb = spool.tile([128, QW], F16)
            qT32 = spool.tile([128, KB], F32)
            srow = spool.tile([1, B * 128], F32)
            t8 = spool.tile([128, B], F32)
            d8 = spool.tile([128, B], F32)
            s8 = spool.tile([128, B], F32)
            e8 = spool.tile([128, B], F32)
            w8u = spool.tile([128, B], F16)
            rZ16 = spool.tile([1, B], F16)
            # block-diagonal softmax weights: wTpad[:, b, b] = w_b, rest 0
            wTpad = spool.tile([128, B, B], F16)
            gAB_sb = spool.tile([128, GW + QW], F16)
            g48f = spool.tile([128, 4 * KB], F32)
            gxt = spool.tile([128, 3 * KB], F32)
            id128 = spool.tile([128, 128], F16)
            rzp = spool.tile([128, 2 * KB], F32)
            rzt = spool.tile([128, 2 * KB], F32)
            hn = spool.tile([128, KB], F32)
            nin = spool.tile([128, KB], F32)
            ngate = spool.tile([128, KB], F32)
            tmpg = spool.tile([128, KB], F32)

            from concourse.masks import make_identity
            nc.gpsimd.memset(ones16[:], 1.0)
            nc.gpsimd.memset(ones128[:], 1.0)
            nc.gpsimd.memset(id1[:], 1.0)
            nc.gpsimd.memset(id1f[:], 1.0)
            nc.gpsimd.memset(wTpad[:], 0.0)
            make_identity(nc, id128[:])
            nc.sync.dma_start(va_sb[:], va_d[:])
            nc.sync.dma_start(bhhn[:], bhhn_d[0][:])

            # ---------------- per-layer prep ----------------
            def prep_layer(l, pp, pspool):
                # UaK = Ua @ keys^T (+ combined bias), per h-tile
                UaT_sb = pp.tile([128, K2H, H], F16, tag="UaT")
                keysT_sb = pp.tile([128, K2H, B * TX], F16, tag="keysT")
                iW_sb = pp.tile([128, KH, H], F16, tag="iW")
                uab_sb = pp.tile([128, KH], F32, tag="uab")
                nc.sync.dma_start(UaT_sb[:], r_kt(UaT_d))
                nc.sync.dma_start(keysT_sb[:], r_kt(keysT_d[l]))
                nc.sync.dma_start(iW_sb[:], r_kt(iW_d[l]))
                nc.sync.dma_start(uab_sb[:], uab_d[:])
                for ht in range(KH):
                    pu = pspool.tile([128, B * TX], F32, tag="pu")
                    for kt in range(K2H):
                        nc.tensor.matmul(pu[:], UaT_sb[:, kt, ht * 128:(ht + 1) * 128],
                                         keysT_sb[:, kt, :], start=(kt == 0),
                                         stop=(kt == K2H - 1))
                    nc.vector.tensor_scalar_add(UaK[:, ht, :], pu[:],
                                                uab_sb[:, ht:ht + 1])
                # s0 = keys[:,0,H:] @ iW  -> h32 (transposed layout)
                for ht in range(KH):
                    ps0 = pspool.tile([128, B], F32, tag="ps0")
                    for kt in range(KH):
                        rhs = keysT_sb[:, KH + kt, :].rearrange(
                            "k (b t) -> k b t", b=B)[:, :, 0]
                        nc.tensor.matmul(ps0[:], iW_sb[:, kt, ht * 128:(ht + 1) * 128],
                                         rhs, start=(kt == 0), stop=(kt == KH - 1))
                    nc.vector.tensor_copy(h32[:, ht * 2:(ht + 1) * 2], ps0[:])
                # KWic[b] = keys[b] @ WicT(pre-scaled): lhsT=keysT slice [128,TX].
                # Wic streamed in 2 column-halves to bound SBUF.
                HW2 = H3 // 2
                for half in range(2):
                    wic_sb = pp.tile([128, K2H, HW2], F16, tag="wic")
                    nc.sync.dma_start(
                        wic_sb[:], r_kt(WicT_d[l])[:, :, half * HW2:(half + 1) * HW2])
                    for b in range(B):
                        for nchunk in range(HW2 // 512):
                            col0 = nchunk * 512
                            pk = pspool.tile([128, 512], F32, tag="pkwic")
                            for kt in range(K2H):
                                nc.tensor.matmul(
                                    pk[:],
                                    keysT_sb[:, kt, b * TX:(b + 1) * TX],
                                    wic_sb[:, kt, col0:col0 + 512],
                                    start=(kt == 0), stop=(kt == K2H - 1))
                            nc.vector.tensor_copy(
                                KWic[l][:, b, half * HW2 + col0:half * HW2 + col0 + 512],
                                pk[:])

            def gx_compute(l, rhsT, KD, WixT_t, pp, pspool):
                # transposed: gx block (pc, kt) = WixT-cols.T @ xT  [128, BT]
                gxb_sb = pp.tile([128, 3 * KH], F32, tag="gxb")
                nc.sync.dma_start(gxb_sb[:], gxb_d[l][:])
                for pcg in range(3):
                    for kt in range(KH):
                        j = (kt // 2) * 6 + pcg * 2 + (kt % 2)
                        pgx = pspool.tile([128, BT], F32, tag="pgx")
                        for kd in range(KD):
                            nc.tensor.matmul(pgx[:], WixT_t[:, kd, j * 128:(j + 1) * 128],
                                             rhsT(kd), start=(kd == 0),
                                             stop=(kd == KD - 1))
                        blk = pcg * KH + kt
                        nc.vector.tensor_scalar_add(gx_sb[l][:, blk, :], pgx[:],
                                                    gxb_sb[:, blk:blk + 1])

            # ---------------- the scan ----------------
            def scan_layer(l, WaT, WhhT, ps):
                pq = ps.tile([128, QW], F32, tag="pq", name=f"pq{l}")
                pg = ps.tile([128, GW + QW], F32, tag="pg", name=f"pg{l}")
                psc = ps.tile([128, 272], F32, tag="psc", name=f"psc{l}")
                ptr = ps.tile([128, 8, 128], F16, tag="ptr", name=f"ptr{l}")
                # dummy-init full tiles so evacuation reads see owned data
                for nnn in range(0, QW, 256):
                    nc.tensor.matmul(pq[:, nnn:nnn + 256], ones16[0:1, 0:128],
                                     ones16[0:1, 0:256], start=True, stop=True)
                for nnn in range(0, GW + QW, 256):
                    nc.tensor.matmul(pg[:, nnn:nnn + 256], ones16[0:1, 0:128],
                                     ones16[0:1, 0:256], start=True, stop=True)
                nc.vector.tensor_copy(h16i[:], h32[:])
                h8f = h8[:].rearrange("p kt b -> p (kt b)")
                if FP8:
                    nc.scalar.mul(h8f, h32[:], HS)
                for t in range(T):
                    def hsl(kt, _t=t):
                        if _t == 0:
                            return h16i[:, kt * 2:kt * 2 + 2]
                        return hsT[l][:, kt, _t - 1, :]
                    if FP8:
                        # q + gh as fp8 DoubleRow streams (K packed in pairs)
                        for kg in range(KH // 2):
                            for g in range(NG):
                                nc.tensor.matmul(
                                    pq[32 * g:32 * g + 2, :],
                                    h8[:, 2 * kg:2 * kg + 2, :],
                                    WaT[:, 2 * kg:2 * kg + 2, g * QW:(g + 1) * QW],
                                    start=(kg == 0), stop=(kg == KH // 2 - 1),
                                    perf_mode=DR,
                                    tile_position=(0, 32 * g), skip_group_check=True)
                        for kg in range(KH // 2):
                            for g in range(NG):
                                nc.tensor.matmul(
                                    pg[32 * g:32 * g + 2, 0:2 * QW],
                                    h8[:, 2 * kg:2 * kg + 2, :],
                                    WhhT[:, 2 * kg:2 * kg + 2,
                                         g * GW:g * GW + 2 * QW],
                                    start=(kg == 0), stop=False,
                                    perf_mode=DR,
                                    tile_position=(0, 32 * g), skip_group_check=True)
                                nc.tensor.matmul(
                                    pg[32 * g:32 * g + 2, GW:GW + QW],
                                    h8[:, 2 * kg:2 * kg + 2, :],
                                    WhhT[:, 2 * kg:2 * kg + 2,
                                         g * GW + 2 * QW:(g + 1) * GW],
                                    start=(kg == 0), stop=(kg == KH // 2 - 1),
                                    perf_mode=DR,
                                    tile_position=(0, 32 * g), skip_group_check=True)
                    else:
                        for kt in range(KH):
                            for g in range(NG):
                                nc.tensor.matmul(
                                    pq[32 * g:32 * g + 2, :], hsl(kt),
                                    WaT[:, kt, g * QW:(g + 1) * QW],
                                    start=(kt == 0), stop=(kt == KH - 1),
                                    tile_position=(0, 32 * g), skip_group_check=True)
                        for kt in range(KH):
                            for g in range(NG):
                                nc.tensor.matmul(
                                    pg[32 * g:32 * g + 2, 0:2 * QW],
                                    hsl(kt),
                                    WhhT[:, kt, g * GW:g * GW + 2 * QW],
                                    start=(kt == 0), stop=False,
                                    tile_position=(0, 32 * g), skip_group_check=True)
                                nc.tensor.matmul(
                                    pg[32 * g:32 * g + 2, GW:GW + QW],
                                    hsl(kt),
                                    WhhT[:, kt, g * GW + 2 * QW:(g + 1) * GW],
                                    start=(kt == 0), stop=(kt == KH - 1),
                                    tile_position=(0, 32 * g), skip_group_check=True)
                    # qT: evac (descale) + PE transpose + strided gather
                    nc.scalar.mul(q_sb[:], pq[:], RSC)
                    for kl in range(2):
                        nc.tensor.transpose(ptr[:, kl, :],
                                            q_sb[:, kl * 128:(kl + 1) * 128],
                                            id128[:])
                    # qT32[p, (2g+kl)*2+b] = ptr[p, kl, 32g+b]
                    gsrc = ptr[:, 0:2, :].rearrange("p kl (g b) -> p kl g b", b=32)[
                        :, :, :, 0:2]
                    gdst = qT32[:].rearrange("p (g kl b) -> p kl g b", kl=2, g=NG)
                    nc.vector.tensor_copy(gdst, gsrc)
                    # A = tanh(UaK + qT): adds on vector, two big tanhs on scalar
                    for ht in range(KH):
                        for b in range(B):
                            nc.vector.tensor_scalar_add(
                                A16[:, ht, b * 128:(b + 1) * 128],
                                UaK[:, ht, b * 128:(b + 1) * 128],
                                qT32[:, ht * 2 + b:ht * 2 + b + 1])
                    for half in range(2):
                        Ah = A16[:, half * 4:(half + 1) * 4, :].rearrange(
                            "p a x -> p (a x)")
                        nc.scalar.activation(Ah, Ah, AF.Tanh)
                    # scores: accumulate va.T @ A over h-tiles -> psc [1, B*128]
                    for ht in range(KH):
                        nc.tensor.matmul(
                            psc[0:1, 0:256], va_sb[:, ht:ht + 1], A16[:, ht, :],
                            start=(ht == 0), stop=(ht == KH - 1),
                            skip_group_check=True)
                    # transposed softmax: scores -> [128(tx), B] partitions
                    nc.scalar.copy(srow[0:1, :], psc[0:1, 0:256])
                    for b in range(B):
                        nc.tensor.transpose(psc[:, 256 + b:257 + b],
                                            srow[0:1, b * 128:(b + 1) * 128],
                                            id1f[:])
                    # exp(x) = ((1+t)/(1-t))^4, t = tanh(x/8); tanh-only table
                    nc.scalar.activation(t8[:], psc[:, 256:258], AF.Tanh, scale=0.125)
                    nc.vector.tensor_scalar_sub(d8[:], t8[:], 1.0)      # t-1
                    nc.vector.reciprocal(d8[:], d8[:])                  # 1/(t-1)
                    nc.vector.tensor_scalar_add(s8[:], t8[:], 1.0)      # 1+t
                    nc.vector.tensor_mul(e8[:], d8[:], s8[:])           # -exp(x/4)
                    nc.vector.tensor_mul(e8[:], e8[:], e8[:])           # exp(x/2)
                    nc.vector.tensor_mul(w8u[:], e8[:], e8[:])          # exp(x)
                    # Z via ones-matmul (partition reduce), 1/Z, broadcast
                    nc.tensor.matmul(psc[0:1, 264:266], ones128[:, 0:1], w8u[:],
                                     start=True, stop=True, skip_group_check=True)
                    with nc.allow_low_precision(reason="1/Z fp16 ok for softmax"):
                        nc.vector.reciprocal(rZ16[:], psc[0:1, 264:266])
                    nc.tensor.matmul(psc[:, 268:270], ones16[0:1, 0:128],
                                     rZ16[0:1, :], start=True, stop=True,
                                     skip_group_check=True)
                    # normalized block-diagonal scatter: wTpad[:,b,b] = w_b/Z_b
                    nc.vector.tensor_mul(
                        wTpad[:].rearrange("p a b -> p (a b)")[:, 0:B * B:B + 1],
                        w8u[:], psc[:, 268:270])
                    # gkc: out[b,:] = w_b @ KWic[b] via block-diag lhsT over
                    # K=2*TX (k-tile kt multiplies KWic[b=kt]); M=2 aligned.
                    for kt in range(B):
                        for g in range(NG):
                            nc.tensor.matmul(
                                pg[32 * g:32 * g + 2, 0:2 * QW],
                                wTpad[:, kt, :],
                                KWic[l][:, kt, g * GW:g * GW + 2 * QW],
                                start=False, stop=(kt == B - 1),
                                tile_position=(0, 32 * g), skip_group_check=True)
                            nc.tensor.matmul(
                                pg[32 * g:32 * g + 2, 2 * QW:3 * QW],
                                wTpad[:, kt, :],
                                KWic[l][:, kt, g * GW + 2 * QW:(g + 1) * GW],
                                start=(kt == 0), stop=(kt == B - 1),
                                tile_position=(0, 32 * g), skip_group_check=True)
                    # gates: evac (descale) + PE transposes + strided gathers
                    nc.vector.tensor_scalar_mul(gAB_sb[:], pg[:], RSC)
                    for j in range(8):
                        nc.tensor.transpose(ptr[:, j, :],
                                            gAB_sb[:, j * 128:(j + 1) * 128],
                                            id128[:])
                    # g48f[p, pc*16+(2g+kl)*2+b] = ptr[p, pc*2+kl, 32g+b]
                    for kl in range(2):
                        gsrc2 = ptr[:, :, :].rearrange(
                            "p (pc kl) (g b) -> p kl pc g b", kl=2, b=32)[
                            :, kl, :, :, 0:2]
                        gdst2 = g48f[:].rearrange(
                            "p (pc g kl b) -> p kl pc g b", pc=4, g=NG, kl=2)[:, kl]
                        nc.vector.tensor_copy(gdst2, gsrc2)
                    # gx slice for this t (SBUF-resident)
                    nc.vector.tensor_copy(
                        gxt[:].rearrange("p (blk b) -> p blk b", b=B),
                        gx_sb[l][:, :, 2 * t:2 * t + 2])
                    if c.debug_h and t == 0 and l == 0:
                        nc.sync.dma_start(dbg["dq"][:], qT32[:])
                        nc.sync.dma_start(dbg["dA"][:],
                                          A16[:].rearrange("p k x -> p (k x)"))
                        nc.sync.dma_start(dbg["dpsc"][:], srow[:])
                        nc.sync.dma_start(dbg["dw"][:],
                                          wTpad[:].rearrange("p a b -> p (a b)"))
                        nc.sync.dma_start(dbg["dg"][:], g48f[:])
                        nc.sync.dma_start(dbg["dgx"][:], gxt[:])
                        nc.sync.dma_start(dbg["dgab"][:], gAB_sb[:])
                        nc.sync.dma_start(
                            dbg["dkwic"][:],
                            KWic[l][:, :, 0:64].rearrange("p b x -> p (b x)"))
                    # gates elementwise (fp32), tanh-only activations:
                    # r,z = sigmoid(x) = 0.5*tanh(x/2)+0.5
                    nc.vector.tensor_add(rzp[:], g48f[:, 0:2 * KB], gxt[:, 0:2 * KB])
                    nc.scalar.activation(rzt[:], rzp[:], AF.Tanh, scale=0.5)
                    nc.vector.tensor_add(hn[:], g48f[:, 3 * KB:4 * KB], bhhn[:])
                    # nin = gx_n + gc_n + 0.5*(1+t_r)*hn
                    nc.vector.scalar_tensor_tensor(
                        tmpg[:], rzt[:, 0:KB], 1.0, hn[:], ALU.add, ALU.mult)
                    nc.vector.tensor_add(nin[:], g48f[:, 2 * KB:3 * KB],
                                         gxt[:, 2 * KB:3 * KB])
                    nc.vector.scalar_tensor_tensor(
                        nin[:], tmpg[:], 0.5, nin[:], ALU.mult, ALU.add)
                    nc.scalar.activation(ngate[:], nin[:], AF.Tanh)
                    # h = n + 0.5*(1+t_z)*(h - n)
                    nc.vector.tensor_sub(tmpg[:], h32[:], ngate[:])
                    nc.vector.scalar_tensor_tensor(
                        tmpg[:], rzt[:, KB:2 * KB], 1.0, tmpg[:], ALU.add, ALU.mult)
                    nc.vector.scalar_tensor_tensor(
                        h32[:], tmpg[:], 0.5, ngate[:], ALU.mult, ALU.add)
                    nc.vector.tensor_copy(
                        hsT[l][:, :, t, :],
                        h32[:].rearrange("p (kt b) -> p kt b", b=B))
                    if FP8 and t < T - 1:
                        nc.scalar.mul(h8f, h32[:], HS)
                if c.debug_h:
                    nc.sync.dma_start(
                        hdbg[l][:],
                        hsT[l][:, :, :, :].rearrange("p kt t b -> p (kt t b)"))

            # ================= phases =================
            with tc.tile_pool(name="prep0", bufs=1) as pp, \
                 tc.tile_pool(name="psA", bufs=1, space="PSUM") as psA:
                prep_layer(0, pp, psA)
            with tc.tile_pool(name="gxp0", bufs=1) as pp, \
                 tc.tile_pool(name="psA2", bufs=1, space="PSUM") as psA2:
                WixT0_sb = pp.tile([128, E // 128, H3], F16, tag="Wix")
                xT_sb = pp.tile([128, E // 128, BT], F16, tag="xTs")
                nc.sync.dma_start(WixT0_sb[:], r_kt(WixT0_d))
                nc.sync.dma_start(xT_sb[:], r_kt(xT_d))
                gx_compute(0, lambda kt: xT_sb[:, kt, :], E // 128, WixT0_sb, pp, psA2)

            for l in range(2):
                if l == 1:
                    nc.sync.dma_start(bhhn[:], bhhn_d[1][:])
                    with tc.tile_pool(name="prep1", bufs=1) as pp, \
                         tc.tile_pool(name="psB", bufs=1, space="PSUM") as psB:
                        prep_layer(1, pp, psB)
                    with tc.tile_pool(name="gxp1", bufs=1) as pp, \
                         tc.tile_pool(name="psB2", bufs=1, space="PSUM") as psB2:
                        WixT1_sb = pp.tile([128, KH, H3], F16, tag="Wix1")
                        nc.sync.dma_start(WixT1_sb[:], r_kt(WixT1_d))
                        gx_compute(1, lambda kt: hsT[0][:, kt, :, :].rearrange(
                                       "p t b -> p (t b)"),
                                   KH, WixT1_sb, pp, psB2)
                with tc.tile_pool(name=f"bigw{l}", bufs=1) as bw, \
                     tc.tile_pool(name=f"psS{l}", bufs=1, space="PSUM") as ps:
                    WaT = bw.tile([128, KH, H], F8 if FP8 else F16, tag="WaT")
                    WhhT = bw.tile([128, KH, H3], F8 if FP8 else F16, tag="WhhT")
                    nc.sync.dma_start(WaT[:], r_kt(WaT_d))
                    nc.sync.dma_start(WhhT[:], r_kt(WhhT_d[l]))
                    scan_layer(l, WaT, WhhT, ps)

            # ---- output projection ----
            with tc.tile_pool(name="proj", bufs=3) as proj, \
                 tc.tile_pool(name="psP", bufs=2, space="PSUM") as psP:
                skipT = spool.tile([128, T * KB], F16, tag="skipT")
                nc.vector.tensor_add(
                    skipT[:],
                    hsT[0][:, :, :, :].rearrange("p kt t b -> p (kt t b)"),
                    hsT[1][:, :, :, :].rearrange("p kt t b -> p (kt t b)"))
                sk3 = skipT[:].rearrange("p (kt tb) -> p kt tb", kt=KH)
                NCH = (V + c.VC - 1) // c.VC
                for nci in range(NCH):
                    n0 = nci * c.VC
                    n1 = min(V, n0 + c.VC)
                    wchunk = proj.tile([128, KH, c.VC], F16, tag="wchunk")
                    nc.sync.dma_start(wchunk[:, :, 0:n1 - n0],
                                      r_kt(outwT_d)[:, :, n0:n1])
                    obc = proj.tile([1, c.VC], F16, tag="obc")
                    nc.sync.dma_start(obc[0:1, 0:n1 - n0], outb_d[0:1, n0:n1])
                    po = psP.tile([128, c.VC], F32, tag="pout")
                    for kt in range(KH):
                        nc.tensor.matmul(po[0:BT, 0:n1 - n0],
                                         sk3[:, kt, :],
                                         wchunk[:, kt, 0:n1 - n0],
                                         start=(kt == 0), stop=False)
                    nc.tensor.matmul(po[0:BT, 0:n1 - n0], ones16[0:1, 0:BT],
                                     obc[0:1, 0:n1 - n0], start=False, stop=True)
                    ot = proj.tile([128, c.VC], F32, tag="ot")
                    nc.vector.tensor_copy(ot[0:BT, 0:n1 - n0], po[0:BT, 0:n1 - n0])
                    nc.sync.dma_start(out_d[:, n0:n1], ot[0:BT, 0:n1 - n0])

    return nc


# ---------------------------------------------------------------------------
def _perm_cols(W3, NG, H):
    """[K, 3H] cols from (gate, h) to (group, gate, h-slice) order."""
    K = W3.shape[0]
    return np.ascontiguousarray(
        W3.reshape(K, 3, NG, H // NG).transpose(0, 2, 1, 3)).reshape(K, 3 * H)


def host_prep(inputs, c: Cfg):
    import ml_dtypes
    f32 = lambda x: np.asarray(x, np.float32)
    f16 = lambda x: np.ascontiguousarray(np.asarray(x, np.float32).astype(np.float16))
    f8c = lambda x: np.ascontiguousarray(
        np.asarray(x, np.float32).astype(ml_dtypes.float8_e4m3))
    fw = f8c if FP8 else f16
    WS = SC / HS if FP8 else SC
    H, E, T, TX, V, NG, B = c.H, c.E, c.T, c.TX, c.V, c.NG, c.B

    emb = f32(inputs["embedding"])
    x_t = np.asarray(inputs["x_t"]).astype(np.int64)[:, :T]
    va = f32(inputs["Va_w"])[0]
    shared = {
        "WaT": fw(f32(inputs["Wa_w"]).T * WS),
        "UaT": f16(f32(inputs["Ua_w"]).T),
        "va": f16(va.reshape(c.KH, 128).T),
        "uab": np.ascontiguousarray(
            (f32(inputs["Ua_b"]) + f32(inputs["Wa_b"])).reshape(c.KH, 128).T
        ).astype(np.float32),
        "outwT": f16(f32(inputs["out_w"]).T[:, :V]),
        "outb": f16(f32(inputs["out_b"])[None, :V]),
        "ones16": np.ones((1, 256), np.float16),
    }
    for l in range(2):
        Wih = f32(inputs[f"gru{l}_Wih"]); Whh = f32(inputs[f"gru{l}_Whh"])
        bih = f32(inputs[f"gru{l}_bih"]); bhh = f32(inputs[f"gru{l}_bhh"])
        Din = Wih.shape[1] - 2 * H
        shared[f"WicT{l}"] = f16(
            _perm_cols(np.ascontiguousarray(Wih[:, Din:].T), NG, H) * SC)
        shared[f"WhhT{l}"] = fw(
            _perm_cols(np.ascontiguousarray(Whh.T), NG, H) * WS)
        gxbv = _perm_cols((bih + np.concatenate(
            [bhh[:2 * H], np.zeros(H, np.float32)]))[None, :], NG, H)[0]
        # block order (pc, kt): j = (kt//2)*6 + pc*2 + kt%2
        gxbT = np.zeros((128, 3 * c.KH), np.float32)
        for pcg in range(3):
            for kt in range(c.KH):
                j = (kt // 2) * 6 + pcg * 2 + (kt % 2)
                gxbT[:, pcg * c.KH + kt] = gxbv[j * 128:(j + 1) * 128]
        shared[f"gxb{l}"] = gxbT
        bn = bhh[2 * H:].reshape(c.KH, 128).T          # [128, KH]
        shared[f"bhhn{l}"] = np.ascontiguousarray(
            np.repeat(bn[:, :, None], B, axis=2).reshape(128, 2 * c.KH)
        ).astype(np.float32)
        shared[f"iW{l}"] = f16(f32(inputs["initialWs"])[l])
        W = f16(_perm_cols(np.ascontiguousarray(Wih[:, :Din].T), NG, H))
        shared["WixT0" if l == 0 else "WixT1"] = W

    ahe = f32(inputs["all_hidden_encoder"])
    in_maps = []
    for core in range(8):
        rows = [2 * core, 2 * core + 1]
        m = dict(shared)
        xe = emb[x_t[rows]]
        m["xT"] = f16(xe.transpose(2, 1, 0).reshape(E, B * T))
        for l in range(2):
            k = ahe[l, rows, :TX]
            m[f"keysT{l}"] = f16(k.transpose(2, 0, 1).reshape(2 * H, B * TX))
        in_maps.append(m)
    return in_maps


_NC_CACHE = {}


def kernel(**inputs) -> np.ndarray:
    c = FULL
    if "nc" not in _NC_CACHE:
        _NC_CACHE["nc"] = build_kernel(c)
    res = run_bass_kernel_spmd(_NC_CACHE["nc"], host_prep(inputs, c),
                               core_ids=list(range(8)))
    outs = []
    for core in range(8):
        o = res.results[core]["out"].reshape(c.T, c.B, c.V).transpose(1, 0, 2)
        outs.append(o)
    return np.concatenate(outs, axis=0).astype(np.float32)
